# revision 12
# baseline (speedup 1.0000x reference)
"""Trainium2 Bass kernel for a custom LSTM cell.

reference:
    z = concat([h_tm1, inputs], -1) @ kernel      # [B, 4U]
    i, f, g, o = split(z, 4, -1)
    c = sigmoid(f) * c_tm1 + sigmoid(i) * tanh(g)
    h = sigmoid(o) * tanh(c)
    returns (h, c)

Sharding over 8 NeuronCores: 2-way over batch x 4-way over units
(each gate's 256-col block co-located per core).  Per core the job is a
[1024, 1536] @ [1536, 1024] GEMM (z) + elementwise epilogue.

Implementation notes (from trace analysis of the fp32r baseline):
  - fp32r moving operands stream at ~2 cycles/elem on HW (389 ns per
    N=512 matmul warm); fp16 streams 1 elem/cycle (~195 ns) at the same
    numerical margin (rel err ~2e-3 vs 2e-2 budget).  All matmul inputs
    are cast to fp16 on host; PSUM accumulation and epilogue stay fp32.
  - The PE is HAM-throttled (K=4/8) until it has been busy for a few
    us; a run of junk warmup matmuls ahead of the real stream starts
    the ramp during the DMA load window.
  - Inputs stream on two HWDGE queues (sync: at, vector: wk) so k-slice
    arrival outpaces the matmul stream from the start.
  - Schedule: wave A = m0..3 x (lo=i|f, hi=g|o) k-outer round-robin over
    8 PSUM banks, paced by DMA arrival; wave B = m4..7 m-serial from
    SBUF-resident data.  Per-m epilogue (ActE sigmoid/tanh + DVE
    combine) drains banks for wave B and pipelines with the stream.
    h and c are packed into one [128, 512] tile per m and leave as one
    DMA on the gpsimd queue.
"""

import sys

sys.path.insert(0, "/opt/trn_rl_repo")

import numpy as np

BATCH, INPUT_DIM, UNITS = 2048, 512, 1024
K = UNITS + INPUT_DIM  # contraction dim, 1536
R, C = 2, 4  # batch halves x unit quarters
BR = BATCH // R  # 1024 batch rows per core
UC = UNITS // C  # 256 units per core
KS = K // 128  # 12 k-subtiles
M = BR // 128  # 8 batch sub-chunks per core
N_WARM = 44  # junk matmuls to start the HAM ramp

_CACHE = {}


def _build_nc():
    import concourse.tile as tile
    from concourse import bacc, mybir

    f32 = mybir.dt.float32
    f16 = mybir.dt.float16
    Sig = mybir.ActivationFunctionType.Sigmoid
    Tanh = mybir.ActivationFunctionType.Tanh

    nc = bacc.Bacc("TRN2")
    at_in = nc.declare_dram_parameter("at", [K, BR], f16, isOutput=False)
    wk_in = nc.declare_dram_parameter("wk", [K, 4 * UC], f16, isOutput=False)
    ct_in = nc.declare_dram_parameter("ct", [BR, UC], f32, isOutput=False)
    hc_out = nc.declare_dram_parameter("hc", [BR, 2 * UC], f32, isOutput=True)

    with tile.TileContext(nc) as tc:
        with (
            tc.tile_pool(name="data", bufs=1) as data,
            tc.tile_pool(name="work", bufs=4) as work,
            tc.tile_pool(name="psum", bufs=8, space="PSUM") as psum,
        ):
            at = data.tile([128, KS, BR], f16)
            wk = data.tile([128, KS, 4 * UC], f16)
            ct = data.tile([128, M, UC], f32)
            sig_if = data.tile([128, M, 2 * UC], f32)
            hc = data.tile([128, M, 2 * UC], f32)
            junk = data.tile([128, 256], f16)

            at_r = at_in[:].rearrange("(ko p) n -> p ko n", p=128)
            wk_r = wk_in[:].rearrange("(ko p) n -> p ko n", p=128)
            ct_r = ct_in[:].rearrange("(m p) u -> p m u", p=128)

            # --- input stream -------------------------------------------
            # One HWDGE queue (sync) in strict consumption order: the SDMA
            # fabric completes roughly in submission order, so a single
            # ordered queue keeps data-arrival matched to the matmul
            # stream.  (Parallel queues complete out of order and stall
            # the PE on the one tensor that lands last.)
            nc.sync.dma_start(at[:, 0:1, 0:128], at_r[:, 0:1, 0:128])
            nc.sync.dma_start(wk[:, 0:1, 0:512], wk_r[:, 0:1, 0:512])
            nc.sync.dma_start(at[:, 0:1, 128:512], at_r[:, 0:1, 128:512])
            nc.sync.dma_start(wk[:, 0:1, 512:1024], wk_r[:, 0:1, 512:1024])
            for k in range(1, KS):
                nc.sync.dma_start(at[:, k : k + 1, 0:512], at_r[:, k : k + 1, 0:512])
                nc.sync.dma_start(wk[:, k : k + 1, :], wk_r[:, k : k + 1, :])
            # wave B's at half streams after everything wave A needs,
            # m-major (wave B consumes all k of one m at once); ct slices
            # land just before the wave-A epilogue needs them.
            nc.sync.dma_start(at[:, :, 512:640], at_r[:, :, 512:640])
            nc.sync.dma_start(ct[:, 0:4, :], ct_r[:, 0:4, :])
            nc.sync.dma_start(at[:, :, 640:768], at_r[:, :, 640:768])
            nc.sync.dma_start(ct[:, 4:8, :], ct_r[:, 4:8, :])
            nc.sync.dma_start(at[:, :, 768:896], at_r[:, :, 768:896])
            nc.sync.dma_start(at[:, :, 896:1024], at_r[:, :, 896:1024])
            nc.vector.memset(junk[:], 0.21484375)

            # --- PE warmup: junk matmuls start the HAM power ramp -------
            warm_ps = psum.tile([128, 512], f32, tag="ps", name="warm_ps")
            for w in range(N_WARM):
                nc.tensor.matmul(
                    warm_ps[:, 0:128],
                    junk[:, 0:128],
                    junk[:, 128:256],
                    start=True,
                    stop=True,
                )

            # --- wave A: m0..3, k-outer round-robin, DMA-paced ----------
            plo = [
                psum.tile([128, 512], f32, tag="ps", name=f"plo{m}") for m in range(4)
            ]
            phi = [
                psum.tile([128, 512], f32, tag="ps", name=f"phi{m}") for m in range(4)
            ]
            for k in range(KS):
                for m in range(4):
                    ms = slice(m * 128, (m + 1) * 128)
                    nc.tensor.matmul(
                        plo[m][:],
                        at[:, k, ms],
                        wk[:, k, 0:512],
                        start=(k == 0),
                        stop=(k == KS - 1),
                    )
                    nc.tensor.matmul(
                        phi[m][:],
                        at[:, k, ms],
                        wk[:, k, 512:1024],
                        start=(k == 0),
                        stop=(k == KS - 1),
                    )

            # wave A epilogue pass 1: activations + c combine
            tg = [None] * M
            so = [None] * M
            for m in range(4):
                nc.scalar.activation(sig_if[:, m, :], plo[m][:], Sig)
                tg[m] = work.tile([128, UC], f32, tag="tg", name=f"tg{m}")
                nc.scalar.activation(tg[m][:], phi[m][:, 0:UC], Tanh)
                so[m] = work.tile([128, UC], f32, tag="so", name=f"so{m}")
                nc.scalar.activation(so[m][:], phi[m][:, UC : 2 * UC], Sig)
                fc = work.tile([128, UC], f32, tag="fc", name=f"fc{m}")
                nc.vector.tensor_mul(fc[:], sig_if[:, m, UC : 2 * UC], ct[:, m, :])
                ig = work.tile([128, UC], f32, tag="ig", name=f"ig{m}")
                nc.vector.tensor_mul(ig[:], sig_if[:, m, 0:UC], tg[m][:])
                nc.vector.tensor_add(hc[:, m, UC : 2 * UC], fc[:], ig[:])
            # wave A epilogue pass 2: h = sig(o) * tanh(c), ship h|c
            for m in range(4):
                th = work.tile([128, UC], f32, tag="th", name=f"th{m}")
                nc.scalar.activation(th[:], hc[:, m, UC : 2 * UC], Tanh)
                nc.vector.tensor_mul(hc[:, m, 0:UC], so[m][:], th[:])
                ms = slice(m * 128, (m + 1) * 128)
                nc.sync.dma_start(hc_out[ms, :], hc[:, m, :])

            # --- wave B: m4..7, m-serial from SBUF-resident data --------
            for m in range(4, M - 1):
                ms = slice(m * 128, (m + 1) * 128)
                plo_b = psum.tile([128, 512], f32, tag="ps", name=f"plo{m}")
                for k in range(KS):
                    nc.tensor.matmul(
                        plo_b[:],
                        at[:, k, ms],
                        wk[:, k, 0:512],
                        start=(k == 0),
                        stop=(k == KS - 1),
                    )
                nc.scalar.activation(sig_if[:, m, :], plo_b[:], Sig)
                phi_b = psum.tile([128, 512], f32, tag="ps", name=f"phi{m}")
                for k in range(KS):
                    nc.tensor.matmul(
                        phi_b[:],
                        at[:, k, ms],
                        wk[:, k, 512:1024],
                        start=(k == 0),
                        stop=(k == KS - 1),
                    )
                tg[m] = work.tile([128, UC], f32, tag="tg", name=f"tg{m}")
                nc.scalar.activation(tg[m][:], phi_b[:, 0:UC], Tanh)
                so[m] = work.tile([128, UC], f32, tag="so", name=f"so{m}")
                nc.scalar.activation(so[m][:], phi_b[:, UC : 2 * UC], Sig)
                fc = work.tile([128, UC], f32, tag="fc", name=f"fc{m}")
                nc.vector.tensor_mul(fc[:], sig_if[:, m, UC : 2 * UC], ct[:, m, :])
                ig = work.tile([128, UC], f32, tag="ig", name=f"ig{m}")
                nc.vector.tensor_mul(ig[:], sig_if[:, m, 0:UC], tg[m][:])
                nc.vector.tensor_add(hc[:, m, UC : 2 * UC], fc[:], ig[:])
                th = work.tile([128, UC], f32, tag="th", name=f"th{m}")
                nc.scalar.activation(th[:], hc[:, m, UC : 2 * UC], Tanh)
                nc.vector.tensor_mul(hc[:, m, 0:UC], so[m][:], th[:])
                nc.sync.dma_start(hc_out[ms, :], hc[:, m, :])

            # last m: lo first (sigmoid/f*c hide under later blocks), then
            # g (tanh chain hides under o block), o last -> short tail:
            # only sig(o) -> h -> dma after the final matmul.
            m = M - 1
            ms = slice(m * 128, (m + 1) * 128)
            plo_b = psum.tile([128, 512], f32, tag="ps", name=f"plo{m}")
            for k in range(KS):
                nc.tensor.matmul(
                    plo_b[:],
                    at[:, k, ms],
                    wk[:, k, 0:512],
                    start=(k == 0),
                    stop=(k == KS - 1),
                )
            nc.scalar.activation(sig_if[:, m, :], plo_b[:], Sig)
            fc = work.tile([128, UC], f32, tag="fc", name=f"fc{m}")
            nc.vector.tensor_mul(fc[:], sig_if[:, m, UC : 2 * UC], ct[:, m, :])
            pg = psum.tile([128, 256], f32, tag="ps", name="pg7")
            for k in range(KS):
                nc.tensor.matmul(
                    pg[:],
                    at[:, k, ms],
                    wk[:, k, 512:768],
                    start=(k == 0),
                    stop=(k == KS - 1),
                )
            tg[m] = work.tile([128, UC], f32, tag="tg", name=f"tg{m}")
            nc.scalar.activation(tg[m][:], pg[:], Tanh)
            ig = work.tile([128, UC], f32, tag="ig", name=f"ig{m}")
            nc.vector.tensor_mul(ig[:], sig_if[:, m, 0:UC], tg[m][:])
            nc.vector.tensor_add(hc[:, m, UC : 2 * UC], fc[:], ig[:])
            th = work.tile([128, UC], f32, tag="th", name=f"th{m}")
            nc.scalar.activation(th[:], hc[:, m, UC : 2 * UC], Tanh)
            nc.sync.dma_start(hc_out[ms, UC : 2 * UC], hc[:, m, UC : 2 * UC])
            po = psum.tile([128, 256], f32, tag="ps", name="po7")
            for k in range(KS):
                nc.tensor.matmul(
                    po[:],
                    at[:, k, ms],
                    wk[:, k, 768:1024],
                    start=(k == 0),
                    stop=(k == KS - 1),
                )
            so[m] = work.tile([128, UC], f32, tag="so", name=f"so{m}")
            nc.scalar.activation(so[m][:], po[:], Sig)
            nc.vector.tensor_mul(hc[:, m, 0:UC], so[m][:], th[:])
            nc.sync.dma_start(hc_out[ms, 0:UC], hc[:, m, 0:UC])

    nc.compile()
    return nc


def get_nc():
    if "nc" not in _CACHE:
        _CACHE["nc"] = _build_nc()
    return _CACHE["nc"]


def make_in_maps(inputs, h_tm1, c_tm1, kernel):
    x = np.asarray(inputs, dtype=np.float32)
    h = np.asarray(h_tm1, dtype=np.float32)
    c = np.asarray(c_tm1, dtype=np.float32)
    w = np.asarray(kernel, dtype=np.float32)
    at_full = np.ascontiguousarray(
        np.concatenate([h, x], axis=1).T.astype(np.float16)
    )  # [K, B] fp16
    in_maps = []
    for core in range(R * C):
        r, ci = divmod(core, C)
        at_np = np.ascontiguousarray(at_full[:, r * BR : (r + 1) * BR])
        gates = [
            w[:, g * UNITS + ci * UC : g * UNITS + (ci + 1) * UC] for g in range(4)
        ]
        wk_np = np.ascontiguousarray(
            np.concatenate(gates, axis=1).astype(np.float16)
        )
        ct_np = np.ascontiguousarray(c[r * BR : (r + 1) * BR, ci * UC : (ci + 1) * UC])
        in_maps.append({"at": at_np, "wk": wk_np, "ct": ct_np})
    return in_maps


def assemble(results):
    h_new = np.empty((BATCH, UNITS), dtype=np.float32)
    c_new = np.empty((BATCH, UNITS), dtype=np.float32)
    for core in range(R * C):
        r, ci = divmod(core, C)
        hc = results[core]["hc"]
        h_new[r * BR : (r + 1) * BR, ci * UC : (ci + 1) * UC] = hc[:, 0:UC]
        c_new[r * BR : (r + 1) * BR, ci * UC : (ci + 1) * UC] = hc[:, UC : 2 * UC]
    return h_new, c_new


def kernel(inputs, h_tm1, c_tm1, kernel):
    from concourse.bass_utils import run_bass_kernel_spmd

    nc = get_nc()
    in_maps = make_in_maps(inputs, h_tm1, c_tm1, kernel)
    res = run_bass_kernel_spmd(nc, in_maps, list(range(R * C)), trace=False)
    return assemble(res.results)


# revision 13
# speedup vs baseline: 1.0281x; 1.0281x over previous
"""Trainium2 Bass kernel for a custom LSTM cell.

reference:
    z = concat([h_tm1, inputs], -1) @ kernel      # [B, 4U]
    i, f, g, o = split(z, 4, -1)
    c = sigmoid(f) * c_tm1 + sigmoid(i) * tanh(g)
    h = sigmoid(o) * tanh(c)
    returns (h, c)

Sharding over 8 NeuronCores: 2-way over batch x 4-way over units
(each gate's 256-col block co-located per core).  Per core the job is a
[1024, 1536] @ [1536, 1024] GEMM (z) + elementwise epilogue.

Implementation notes (from trace analysis of the fp32r baseline):
  - fp32r moving operands stream at ~2 cycles/elem on HW (389 ns per
    N=512 matmul warm); fp16 streams 1 elem/cycle (~195 ns) at the same
    numerical margin (rel err ~2e-3 vs 2e-2 budget).  All matmul inputs
    are cast to fp16 on host; PSUM accumulation and epilogue stay fp32.
  - The PE is HAM-throttled (K=4/8) until it has been busy for a few
    us; a run of junk warmup matmuls ahead of the real stream starts
    the ramp during the DMA load window.
  - Inputs stream on two HWDGE queues (sync: at, vector: wk) so k-slice
    arrival outpaces the matmul stream from the start.
  - Schedule: wave A = m0..3 x (lo=i|f, hi=g|o) k-outer round-robin over
    8 PSUM banks, paced by DMA arrival; wave B = m4..7 m-serial from
    SBUF-resident data.  Per-m epilogue (ActE sigmoid/tanh + DVE
    combine) drains banks for wave B and pipelines with the stream.
    h and c are packed into one [128, 512] tile per m and leave as one
    DMA on the gpsimd queue.
"""

import sys

sys.path.insert(0, "/opt/trn_rl_repo")

import numpy as np

BATCH, INPUT_DIM, UNITS = 2048, 512, 1024
K = UNITS + INPUT_DIM  # contraction dim, 1536
R, C = 2, 4  # batch halves x unit quarters
BR = BATCH // R  # 1024 batch rows per core
UC = UNITS // C  # 256 units per core
KS = K // 128  # 12 k-subtiles
M = BR // 128  # 8 batch sub-chunks per core
N_WARM = 36  # junk matmuls to start the HAM ramp

_CACHE = {}


def _build_nc():
    import concourse.tile as tile
    from concourse import bacc, mybir

    f32 = mybir.dt.float32
    f16 = mybir.dt.float16
    Sig = mybir.ActivationFunctionType.Sigmoid
    Tanh = mybir.ActivationFunctionType.Tanh

    nc = bacc.Bacc("TRN2")
    at_in = nc.declare_dram_parameter("at", [K, BR], f16, isOutput=False)
    wk_in = nc.declare_dram_parameter("wk", [K, 4 * UC], f16, isOutput=False)
    ct_in = nc.declare_dram_parameter("ct", [BR, UC], f32, isOutput=False)
    hc_out = nc.declare_dram_parameter("hc", [BR, 2 * UC], f32, isOutput=True)

    with tile.TileContext(nc) as tc:
        with (
            tc.tile_pool(name="data", bufs=1) as data,
            tc.tile_pool(name="work", bufs=4) as work,
            tc.tile_pool(name="psum", bufs=8, space="PSUM") as psum,
        ):
            at = data.tile([128, KS, BR], f16)
            wk = data.tile([128, KS, 4 * UC], f16)
            ct = data.tile([128, M, UC], f32)
            sig_if = data.tile([128, M, 2 * UC], f32)
            hc = data.tile([128, M, 2 * UC], f32)
            junk = data.tile([128, 256], f16)

            at_r = at_in[:].rearrange("(ko p) n -> p ko n", p=128)
            wk_r = wk_in[:].rearrange("(ko p) n -> p ko n", p=128)
            ct_r = ct_in[:].rearrange("(m p) u -> p m u", p=128)

            # --- input stream -------------------------------------------
            # One HWDGE queue (sync) in strict consumption order: the SDMA
            # fabric completes roughly in submission order, so a single
            # ordered queue keeps data-arrival matched to the matmul
            # stream.  (Parallel queues complete out of order and stall
            # the PE on the one tensor that lands last.)
            nc.sync.dma_start(at[:, 0:1, 0:128], at_r[:, 0:1, 0:128])
            nc.sync.dma_start(wk[:, 0:1, 0:512], wk_r[:, 0:1, 0:512])
            nc.sync.dma_start(at[:, 0:1, 128:512], at_r[:, 0:1, 128:512])
            nc.sync.dma_start(wk[:, 0:1, 512:1024], wk_r[:, 0:1, 512:1024])
            for k in range(1, KS):
                nc.sync.dma_start(at[:, k : k + 1, 0:512], at_r[:, k : k + 1, 0:512])
                nc.sync.dma_start(wk[:, k : k + 1, :], wk_r[:, k : k + 1, :])
            # wave B's at half streams after everything wave A needs,
            # m-major (wave B consumes all k of one m at once); ct slices
            # land just before the wave-A epilogue needs them.
            nc.sync.dma_start(at[:, :, 512:640], at_r[:, :, 512:640])
            nc.sync.dma_start(ct[:, 0:4, :], ct_r[:, 0:4, :])
            nc.sync.dma_start(at[:, :, 640:768], at_r[:, :, 640:768])
            nc.sync.dma_start(ct[:, 4:8, :], ct_r[:, 4:8, :])
            nc.sync.dma_start(at[:, :, 768:896], at_r[:, :, 768:896])
            nc.sync.dma_start(at[:, :, 896:1024], at_r[:, :, 896:1024])
            nc.vector.memset(junk[:], 0.21484375)

            # --- PE warmup: junk matmuls start the HAM power ramp -------
            warm_ps = psum.tile([128, 512], f32, tag="ps", name="warm_ps")
            for w in range(N_WARM):
                nc.tensor.matmul(
                    warm_ps[:, 0:128],
                    junk[:, 0:128],
                    junk[:, 128:256],
                    start=True,
                    stop=True,
                )

            # --- wave A: m0..3, k-outer round-robin, DMA-paced ----------
            plo = [
                psum.tile([128, 512], f32, tag="ps", name=f"plo{m}") for m in range(4)
            ]
            phi = [
                psum.tile([128, 512], f32, tag="ps", name=f"phi{m}") for m in range(4)
            ]
            for k in range(KS):
                for m in range(4):
                    ms = slice(m * 128, (m + 1) * 128)
                    nc.tensor.matmul(
                        plo[m][:],
                        at[:, k, ms],
                        wk[:, k, 0:512],
                        start=(k == 0),
                        stop=(k == KS - 1),
                    )
                    nc.tensor.matmul(
                        phi[m][:],
                        at[:, k, ms],
                        wk[:, k, 512:1024],
                        start=(k == 0),
                        stop=(k == KS - 1),
                    )

            # wave A epilogue pass 1: activations + c combine
            tg = [None] * M
            so = [None] * M
            for m in range(4):
                nc.scalar.activation(sig_if[:, m, :], plo[m][:], Sig)
                tg[m] = work.tile([128, UC], f32, tag="tg", name=f"tg{m}")
                nc.scalar.activation(tg[m][:], phi[m][:, 0:UC], Tanh)
                so[m] = work.tile([128, UC], f32, tag="so", name=f"so{m}")
                nc.scalar.activation(so[m][:], phi[m][:, UC : 2 * UC], Sig)
                fc = work.tile([128, UC], f32, tag="fc", name=f"fc{m}")
                nc.vector.tensor_mul(fc[:], sig_if[:, m, UC : 2 * UC], ct[:, m, :])
                ig = work.tile([128, UC], f32, tag="ig", name=f"ig{m}")
                nc.vector.tensor_mul(ig[:], sig_if[:, m, 0:UC], tg[m][:])
                nc.vector.tensor_add(hc[:, m, UC : 2 * UC], fc[:], ig[:])
            # wave A epilogue pass 2: h = sig(o) * tanh(c), ship h|c
            for m in range(4):
                th = work.tile([128, UC], f32, tag="th", name=f"th{m}")
                nc.scalar.activation(th[:], hc[:, m, UC : 2 * UC], Tanh)
                nc.vector.tensor_mul(hc[:, m, 0:UC], so[m][:], th[:])
                ms = slice(m * 128, (m + 1) * 128)
                nc.sync.dma_start(hc_out[ms, :], hc[:, m, :])

            # --- wave B: m4..7, m-serial from SBUF-resident data --------
            for m in range(4, M - 1):
                ms = slice(m * 128, (m + 1) * 128)
                plo_b = psum.tile([128, 512], f32, tag="ps", name=f"plo{m}")
                for k in range(KS):
                    nc.tensor.matmul(
                        plo_b[:],
                        at[:, k, ms],
                        wk[:, k, 0:512],
                        start=(k == 0),
                        stop=(k == KS - 1),
                    )
                nc.scalar.activation(sig_if[:, m, :], plo_b[:], Sig)
                phi_b = psum.tile([128, 512], f32, tag="ps", name=f"phi{m}")
                for k in range(KS):
                    nc.tensor.matmul(
                        phi_b[:],
                        at[:, k, ms],
                        wk[:, k, 512:1024],
                        start=(k == 0),
                        stop=(k == KS - 1),
                    )
                tg[m] = work.tile([128, UC], f32, tag="tg", name=f"tg{m}")
                nc.scalar.activation(tg[m][:], phi_b[:, 0:UC], Tanh)
                so[m] = work.tile([128, UC], f32, tag="so", name=f"so{m}")
                nc.scalar.activation(so[m][:], phi_b[:, UC : 2 * UC], Sig)
                fc = work.tile([128, UC], f32, tag="fc", name=f"fc{m}")
                nc.vector.tensor_mul(fc[:], sig_if[:, m, UC : 2 * UC], ct[:, m, :])
                ig = work.tile([128, UC], f32, tag="ig", name=f"ig{m}")
                nc.vector.tensor_mul(ig[:], sig_if[:, m, 0:UC], tg[m][:])
                nc.vector.tensor_add(hc[:, m, UC : 2 * UC], fc[:], ig[:])
                th = work.tile([128, UC], f32, tag="th", name=f"th{m}")
                nc.scalar.activation(th[:], hc[:, m, UC : 2 * UC], Tanh)
                nc.vector.tensor_mul(hc[:, m, 0:UC], so[m][:], th[:])
                nc.sync.dma_start(hc_out[ms, :], hc[:, m, :])

            # last m: lo first (sigmoid/f*c hide under later blocks), then
            # g (tanh chain hides under o block), o last -> short tail:
            # only sig(o) -> h -> dma after the final matmul.
            m = M - 1
            ms = slice(m * 128, (m + 1) * 128)
            plo_b = psum.tile([128, 512], f32, tag="ps", name=f"plo{m}")
            for k in range(KS):
                nc.tensor.matmul(
                    plo_b[:],
                    at[:, k, ms],
                    wk[:, k, 0:512],
                    start=(k == 0),
                    stop=(k == KS - 1),
                )
            nc.scalar.activation(sig_if[:, m, :], plo_b[:], Sig)
            fc = work.tile([128, UC], f32, tag="fc", name=f"fc{m}")
            nc.vector.tensor_mul(fc[:], sig_if[:, m, UC : 2 * UC], ct[:, m, :])
            pg = psum.tile([128, 256], f32, tag="ps", name="pg7")
            for k in range(KS):
                nc.tensor.matmul(
                    pg[:],
                    at[:, k, ms],
                    wk[:, k, 512:768],
                    start=(k == 0),
                    stop=(k == KS - 1),
                )
            tg[m] = work.tile([128, UC], f32, tag="tg", name=f"tg{m}")
            nc.scalar.activation(tg[m][:], pg[:], Tanh)
            ig = work.tile([128, UC], f32, tag="ig", name=f"ig{m}")
            nc.vector.tensor_mul(ig[:], sig_if[:, m, 0:UC], tg[m][:])
            nc.vector.tensor_add(hc[:, m, UC : 2 * UC], fc[:], ig[:])
            th = work.tile([128, UC], f32, tag="th", name=f"th{m}")
            nc.scalar.activation(th[:], hc[:, m, UC : 2 * UC], Tanh)
            nc.sync.dma_start(hc_out[ms, UC : 2 * UC], hc[:, m, UC : 2 * UC])
            po = psum.tile([128, 256], f32, tag="ps", name="po7")
            for k in range(KS):
                nc.tensor.matmul(
                    po[:],
                    at[:, k, ms],
                    wk[:, k, 768:1024],
                    start=(k == 0),
                    stop=(k == KS - 1),
                )
            so[m] = work.tile([128, UC], f32, tag="so", name=f"so{m}")
            nc.scalar.activation(so[m][:], po[:], Sig)
            nc.vector.tensor_mul(hc[:, m, 0:UC], so[m][:], th[:])
            nc.sync.dma_start(hc_out[ms, 0:UC], hc[:, m, 0:UC])

    nc.compile()
    return nc


def get_nc():
    if "nc" not in _CACHE:
        _CACHE["nc"] = _build_nc()
    return _CACHE["nc"]


def make_in_maps(inputs, h_tm1, c_tm1, kernel):
    x = np.asarray(inputs, dtype=np.float32)
    h = np.asarray(h_tm1, dtype=np.float32)
    c = np.asarray(c_tm1, dtype=np.float32)
    w = np.asarray(kernel, dtype=np.float32)
    at_full = np.ascontiguousarray(
        np.concatenate([h, x], axis=1).T.astype(np.float16)
    )  # [K, B] fp16
    in_maps = []
    for core in range(R * C):
        r, ci = divmod(core, C)
        at_np = np.ascontiguousarray(at_full[:, r * BR : (r + 1) * BR])
        gates = [
            w[:, g * UNITS + ci * UC : g * UNITS + (ci + 1) * UC] for g in range(4)
        ]
        wk_np = np.ascontiguousarray(
            np.concatenate(gates, axis=1).astype(np.float16)
        )
        ct_np = np.ascontiguousarray(c[r * BR : (r + 1) * BR, ci * UC : (ci + 1) * UC])
        in_maps.append({"at": at_np, "wk": wk_np, "ct": ct_np})
    return in_maps


def assemble(results):
    h_new = np.empty((BATCH, UNITS), dtype=np.float32)
    c_new = np.empty((BATCH, UNITS), dtype=np.float32)
    for core in range(R * C):
        r, ci = divmod(core, C)
        hc = results[core]["hc"]
        h_new[r * BR : (r + 1) * BR, ci * UC : (ci + 1) * UC] = hc[:, 0:UC]
        c_new[r * BR : (r + 1) * BR, ci * UC : (ci + 1) * UC] = hc[:, UC : 2 * UC]
    return h_new, c_new


def kernel(inputs, h_tm1, c_tm1, kernel):
    from concourse.bass_utils import run_bass_kernel_spmd

    nc = get_nc()
    in_maps = make_in_maps(inputs, h_tm1, c_tm1, kernel)
    res = run_bass_kernel_spmd(nc, in_maps, list(range(R * C)), trace=False)
    return assemble(res.results)
